# revision 30
# baseline (speedup 1.0000x reference)
"""CE + CJS loss kernel for Trainium2, data-parallel over 8 NeuronCores.

Math (reference):
    logp = log_softmax(pred_logit, axis=1)          # x - lse_i
    ce   = -mean_i( sum_j gt*logp )
    p    = softmax(pred_logit)
    m    = 0.5*(gt + p + EPS)
    contrib = gt*ln(gt) + p*logp - (gt+p)*ln(m)     # per element
    cjs  = 0.5 * sum_ij w_j * contrib_ij / B,  w_j = C - j
    loss = ce + 0.5*cjs

Kernel decomposition (v2):
    With xp = x - lse, u = gt + p, q = xp - logm:
        f1 = gt*lngt, f2 = u*q, f4 = gt*xp
        contrib = f1 + f2 - f4          (exactly)
        CE total = sum_ij f4
    Per-column sums of f1+f2 and of f4 accumulate in two PSUM bank sets
    via ones-vector matmuls; host applies the w_j weighting and the
    subtraction in float64.
    Engine balance per core (~89us HBM roofline):
      ScalarE: Exp(x)+rowsum, Ln(gt), Ln(m)            (3 passes)
      VectorE: p, u, xp, f1, q, f2                     (~2.5 cyc/elem)
      GpSimd:  f4 = gt*xp + the casting input DMAs
      TensorE: 3 colsum streams (24 matmuls / 4096-chunk)
    Inputs are loaded as bf16 via gpsimd casting DMAs (f32 in HBM).
"""
import numpy as np

import concourse.bass as bass
import concourse.tile as tile
from concourse import mybir
from concourse.bass_utils import run_bass_kernel_spmd
from concourse.vector_clock import ScopedClock

B, C = 4096, 8192
N_CORES = 8
ROWS = B // N_CORES          # 512 rows per core
N_BLK = ROWS // 128          # 4 partition blocks
F2 = 4096                    # chunk width
N_CHUNK = C // F2            # 2 chunks per block
NSL = F2 // 512              # 8 matmul slices per chunk
N_SLICE = C // 512           # 16 column slices total
EPS = 1e-8

# config flags (fallbacks for primitives that may not work on HW)
X_CAST_DMA = True    # x via gpsimd casting DMA (f32 HBM -> bf16 SBUF)
GT_CAST_DMA = True   # gt likewise
G_F4 = 0             # chunks computing f4 on GpSimd (HW: SBUF-port contention
                     # with VectorE makes GpSimd elementwise a net loss)
WARM_MM = False      # dummy matmuls can't trigger the HAM's 3.4us sustained-
                     # busy window; they only added matmul count (measured)

f32 = mybir.dt.float32
bf16 = mybir.dt.bfloat16
AF = mybir.ActivationFunctionType
ALU = mybir.AluOpType


def _patched_drain_and_barrier(self, tick_clock, wait_clock):
    # Walrus CoreV3 codegen allows only ONE sync-wait command on a
    # Drain/NoOp (NO_STRUCT ctrl). The stock Tile tail drain carries one
    # wait per pending engine clock and fails to compile. Split the waits
    # across single-wait SP nops; SP executes in program order, so the
    # drain still orders after everything.
    nc = self.nc
    probe = nc.sync.nop().ins
    wait_clock.add_sem_waits(probe, ScopedClock({None: tick_clock.global_clock}))
    waits = list(probe.sync_info.on_wait) if probe.sync_info else []
    probe.sync_info = mybir.SyncInfo(on_wait=waits[:1], on_update=[])
    for w in waits[1:]:
        extra = nc.sync.nop().ins
        extra.sync_info = mybir.SyncInfo(on_wait=[w], on_update=[])
    nc.sync.drain()
    nc.all_engine_barrier()
    assert self.sems is not None
    popped = nc._tile_sem_poison_stack.pop()
    assert popped is self._sem_poison
    nc.clear_and_free_semaphores(list(self.sems.allocated().values()))
    nc.all_engine_barrier()


tile.TileContext._drain_and_barrier = _patched_drain_and_barrier


def _split_excess_waits(nc: bass.Bass, max_waits: int = 1):
    # Same walrus limitation, general form: cap sync waits per instruction,
    # hoisting the excess onto same-engine NOPs inserted just before (the
    # engine executes its stream in order, so semantics are unchanged).
    for bb in nc.main_func.blocks:
        insts = list(bb.instructions)
        out, changed = [], False
        for ins in insts:
            si = ins.sync_info
            waits = list(si.on_wait) if (si is not None and si.on_wait) else []
            if len(waits) > max_waits:
                ups = list(si.on_update) if si.on_update else []
                for w in waits[:-max_waits]:
                    nop = mybir.InstNoOp(
                        name=nc.get_next_instruction_name(), ins=[], outs=[])
                    nop.engine = ins.engine
                    nop.sync_info = mybir.SyncInfo(on_wait=[w], on_update=[])
                    nc.register_instruction(nop)
                    out.append(nop)
                ins.sync_info = mybir.SyncInfo(
                    on_wait=waits[-max_waits:], on_update=ups)
                changed = True
            out.append(ins)
        if changed:
            bb.instructions = out


def build_nc() -> bass.Bass:
    nc = bass.Bass()
    x_dram = nc.declare_dram_parameter("pred_logit", [ROWS, C], f32, isOutput=False)
    gt_dram = nc.declare_dram_parameter("gt", [ROWS, C], f32, isOutput=False)
    cs_dram = nc.declare_dram_parameter("partials", [N_SLICE, 512], f32, isOutput=True)
    f4_dram = nc.declare_dram_parameter("partials_f4", [N_SLICE, 512], f32, isOutput=True)

    from contextlib import ExitStack
    with tile.TileContext(nc) as tc, ExitStack() as es:
        consts = es.enter_context(tc.tile_pool(name="consts", bufs=1))
        xpool = es.enter_context(tc.tile_pool(name="xpool", bufs=2))
        tpool = es.enter_context(tc.tile_pool(name="tpool", bufs=2))
        rowp = es.enter_context(tc.tile_pool(name="rowp", bufs=2))
        gtp = es.enter_context(tc.tile_pool(name="gtp", bufs=4))
        ck = es.enter_context(tc.tile_pool(name="ck", bufs=2))
        psum = es.enter_context(tc.tile_pool(name="psum", bufs=1, space="PSUM"))

        ones = consts.tile([128, 1], bf16)
        nc.vector.memset(ones, 1.0)
        neg_ones = consts.tile([128, 1], bf16)
        nc.vector.memset(neg_ones, -1.0)
        eps_half = consts.tile([128, 1], f32)
        nc.vector.memset(eps_half, 0.5 * EPS)

        # PSUM: two bank sets of 4 banks x 4 base-partitions = 16 column-
        # slice regions each. cs accumulates f1 + f2; f4set accumulates
        # f4 = gt*xp. Host computes w.(cs - f4cs) and ce = sum(f4cs).
        csb = [psum.tile([128, 512], f32, name=f"cs{i}", tag=f"cs{i}")
               for i in range(4)]
        f4b = [psum.tile([128, 512], f32, name=f"f4{i}", tag=f"f4{i}")
               for i in range(4)]

        def cs_mm(m, rhs, start, stop):
            base = 32 * (m % 4)
            nc.tensor.matmul(csb[m // 4][base:base + 1, :], ones[:], rhs,
                             start=start, stop=stop, tile_position=(0, base))

        def f4_mm(m, rhs, start, stop):
            base = 32 * (m % 4)
            nc.tensor.matmul(f4b[m // 4][base:base + 1, :], ones[:], rhs,
                             start=start, stop=stop, tile_position=(0, base))

        def warm_mm(anchor_ap):
            pass  # disabled (see WARM_MM)

        # lookahead-allocated input tiles + their DMAs (gpsimd queue for
        # casting DMAs must be primed a block ahead of the f4 work)
        xdt = bf16 if X_CAST_DMA else f32
        gdt = bf16 if GT_CAST_DMA else f32
        xtiles, gtiles = {}, {}
        N_XSUB = 4
        XS = C // N_XSUB

        def emit_x_dmas(b):
            # quartered so block 0's first exp starts ~3us after launch
            r0 = b * 128
            xb = xpool.tile([128, C], xdt, tag="x")
            xtiles[b] = xb
            eng = nc.gpsimd if X_CAST_DMA else nc.sync
            for h in range(N_XSUB):
                sl = slice(h * XS, (h + 1) * XS)
                eng.dma_start(out=xb[:, sl], in_=x_dram[r0:r0 + 128, sl])

        def emit_gt_dmas(b):
            r0 = b * 128
            eng = nc.gpsimd if GT_CAST_DMA else nc.sync
            for c in range(N_CHUNK):
                sl = slice(c * F2, (c + 1) * F2)
                g = gtp.tile([128, F2], gdt, tag="gt")
                gtiles[(b, c)] = g
                eng.dma_start(out=g[:], in_=gt_dram[r0:r0 + 128, sl])

        emit_x_dmas(0)
        for b in range(N_BLK):
            xb = xtiles[b]
            tb = tpool.tile([128, C], bf16, tag="t")
            # exp split per x-DMA quarter so it starts as soon as one lands
            s4 = rowp.tile([128, N_XSUB], f32, tag="s4")
            for h in range(N_XSUB):
                sl = slice(h * XS, (h + 1) * XS)
                nc.scalar.activation(out=tb[:, sl], in_=xb[:, sl], func=AF.Exp,
                                     accum_out=s4[:, h:h + 1])
            s = rowp.tile([128, 1], f32, tag="s")
            nc.vector.tensor_reduce(out=s[:], in_=s4[:], op=ALU.add,
                                    axis=mybir.AxisListType.X)
            recip = rowp.tile([128, 1], f32, tag="recip")
            nc.vector.reciprocal(out=recip[:], in_=s[:])
            lse = rowp.tile([128, 1], f32, tag="lse")
            nc.scalar.activation(out=lse[:], in_=s[:], func=AF.Ln)

            # gt(0) is deferred behind x(0) on the gpsimd queue so block
            # 0's x transfers get the full HBM bandwidth; later blocks'
            # inputs are primed a block ahead so transfers overlap compute
            if b == 0:
                emit_gt_dmas(0)
            if b + 1 < N_BLK:
                emit_x_dmas(b + 1)
                emit_gt_dmas(b + 1)

            # both chunks' Ln(gt) first so ScalarE never stalls behind logm
            lngts = {}
            for c in range(N_CHUNK):
                g = gtiles[(b, c)]
                lngt = ck.tile([128, F2], bf16, tag="lngt")
                nc.scalar.activation(out=lngt[:], in_=g[:], func=AF.Ln)
                lngts[c] = lngt

            for c in range(N_CHUNK):
                sl = slice(c * F2, (c + 1) * F2)
                g = gtiles[(b, c)]
                gt16 = g
                if not GT_CAST_DMA:
                    gt16 = ck.tile([128, F2], bf16, tag="gt16")
                    nc.vector.tensor_copy(out=gt16[:], in_=g[:])

                first, last = (b == 0), (b == N_BLK - 1)
                p = ck.tile([128, F2], bf16, tag="p", bufs=1)
                nc.vector.tensor_scalar(
                    out=p[:], in0=tb[:, sl], scalar1=recip[:], scalar2=None,
                    op0=ALU.mult)
                warm_mm(p[:, 0:128])
                u = ck.tile([128, F2], bf16, tag="u")
                nc.vector.tensor_tensor(out=u[:], in0=gt16[:], in1=p[:], op=ALU.add)
                xp = ck.tile([128, F2], bf16, tag="xp")
                nc.vector.tensor_scalar(
                    out=xp[:], in0=xb[:, sl], scalar1=lse[:], scalar2=None,
                    op0=ALU.subtract)
                warm_mm(xp[:, 0:128])
                # ScalarE: logm as soon as u lands; VectorE meanwhile does f1
                logm = ck.tile([128, F2], bf16, tag="logm")
                nc.scalar.activation(out=logm[:], in_=u[:], func=AF.Ln,
                                     scale=0.5, bias=eps_half[:])
                f1 = ck.tile([128, F2], bf16, tag="f1", bufs=1)
                nc.vector.tensor_tensor(out=f1[:], in0=gt16[:], in1=lngts[c][:],
                                        op=ALU.mult)
                for k in range(NSL):
                    m = c * NSL + k
                    ksl = slice(k * 512, (k + 1) * 512)
                    cs_mm(m, f1[:, ksl], start=first, stop=False)
                q = ck.tile([128, F2], bf16, tag="q", bufs=1)
                nc.vector.tensor_tensor(out=q[:], in0=xp[:], in1=logm[:],
                                        op=ALU.subtract)
                warm_mm(q[:, 0:128])
                f2 = ck.tile([128, F2], bf16, tag="f2", bufs=1)
                nc.vector.tensor_tensor(out=f2[:], in0=u[:], in1=q[:], op=ALU.mult)
                for k in range(NSL):
                    m = c * NSL + k
                    ksl = slice(k * 512, (k + 1) * 512)
                    cs_mm(m, f2[:, ksl], start=False, stop=last)
                f4 = ck.tile([128, F2], bf16, tag="f4", bufs=1)
                ci = b * N_CHUNK + c
                g_set = {round(i * 8 / G_F4) for i in range(G_F4)} if G_F4 > 0 else set()
                eng = nc.gpsimd if ci in g_set or G_F4 >= 8 else nc.vector
                eng.tensor_tensor(out=f4[:], in0=gt16[:], in1=xp[:], op=ALU.mult)
                for k in range(NSL):
                    m = c * NSL + k
                    ksl = slice(k * 512, (k + 1) * 512)
                    f4_mm(m, f4[:, ksl], start=first, stop=last)

        # PSUM is not DMA-readable: bounce through SBUF (copies split
        # between ScalarE and VectorE), then one partition-strided DMA
        # per bank writes its 4 result rows.
        for i in range(4):
            sb = consts.tile([128, 512], f32, tag="sbounce", bufs=2)
            if i % 2 == 0:
                nc.scalar.copy(out=sb[:], in_=csb[i][:])
            else:
                nc.vector.tensor_copy(out=sb[:], in_=csb[i][:])
            nc.sync.dma_start(out=cs_dram[4 * i:4 * i + 4, :],
                              in_=sb[0:128:32, :])
        for i in range(4):
            sb = consts.tile([128, 512], f32, tag="sbounce", bufs=2)
            if i % 2 == 0:
                nc.vector.tensor_copy(out=sb[:], in_=f4b[i][:])
            else:
                nc.scalar.copy(out=sb[:], in_=f4b[i][:])
            nc.sync.dma_start(out=f4_dram[4 * i:4 * i + 4, :],
                              in_=sb[0:128:32, :])

    _split_excess_waits(nc)
    return nc


_NC_CACHE = None
LAST_EXEC_NS = None
LAST_TRACE = None


def kernel(pred_logit: np.ndarray, gt: np.ndarray) -> np.ndarray:
    global _NC_CACHE, LAST_EXEC_NS, LAST_TRACE
    if _NC_CACHE is None:
        _NC_CACHE = build_nc()
    nc = _NC_CACHE

    pred_logit = np.ascontiguousarray(pred_logit, dtype=np.float32)
    gt = np.ascontiguousarray(gt, dtype=np.float32)
    in_maps = [
        {
            "pred_logit": pred_logit[c * ROWS:(c + 1) * ROWS],
            "gt": gt[c * ROWS:(c + 1) * ROWS],
        }
        for c in range(N_CORES)
    ]
    res = run_bass_kernel_spmd(nc, in_maps, list(range(N_CORES)))
    if res.exec_time_ns is not None:
        LAST_EXEC_NS = res.exec_time_ns
        if res.instructions_and_trace:
            LAST_TRACE = res.instructions_and_trace[1]

    w = (C - np.arange(C)).astype(np.float64)
    e1_total = 0.0   # sum_j w_j * colsum(contrib)_j
    ce_total = 0.0   # sum_ij gt*xp
    for r in res.results:
        cs = r["partials"].astype(np.float64).reshape(C)
        f4cs = r["partials_f4"].astype(np.float64).reshape(C)
        e1_total += np.dot(w, cs - f4cs)
        ce_total += f4cs.sum()
    loss = -ce_total / B + 0.25 * e1_total / B
    return np.array(loss, dtype=np.float32)
